# revision 1
# baseline (speedup 1.0000x reference)
"""Tensor-parallel DeepSpeed-style self-attention block on 8 TRN2 NeuronCores.

Strategy (head-sharded QKV/attention + all-to-all + token-sharded output GEMM):
  - LayerNorm params are folded into the QKV weight/bias on host:
      qkv = z @ (norm_w[:,None]*W) + (norm_b @ W + qkvb),  z=(x-mu)*istd
  - Each core owns 2 of 16 heads: computes z (replicated), z^T via PE
    transposes, then Q^T,K^T (transposed layout) and V (natural layout)
    for its heads over all 4096 tokens.
  - Causal attention per (batch, head): scores = Q^T.T @ K^T in PSUM,
    only key-blocks <= diagonal; triangular mask added to the diagonal
    128x128 block; exp on ScalarE with accum_out row-sums (no max
    subtraction: |scores/sqrt(d)| <= ~6 for this distribution);
    p transposed per 128-chunk on PE; ctx = p^T.T @ V accumulated in
    PSUM; normalized by 1/rowsum during PSUM->SBUF copy; transposed to
    ctx^T.
  - AllToAll (one per batch, bf16, 1MB) redistributes ctx^T from
    head-sharded to token-sharded: each core ends with all 16 heads for
    its 256-token slice of each batch.
  - Output GEMM: full attn_ow (replicated, cast bf16) x token shard.
    Each core writes a [512, 2048] f32 output shard; host concatenates.
"""

import sys

if "/opt/trn_rl_repo" not in sys.path:
    sys.path.insert(0, "/opt/trn_rl_repo")

# --- shim antenv.axon_hooks (missing in this image) so trace=True can NTFF-profile ---
import types, ctypes, contextlib


def _make_ntff_hook(so_path="/opt/axon/libaxon_pjrt.so"):
    try:
        lib = ctypes.CDLL(so_path)
    except OSError:
        return None
    if not hasattr(lib, "axon_start_nrt_profile"):
        return None
    lib.axon_start_nrt_profile.argtypes = [ctypes.POINTER(ctypes.c_int64), ctypes.c_size_t]
    lib.axon_start_nrt_profile.restype = ctypes.c_int64
    lib.axon_stop_nrt_profile.argtypes = [ctypes.c_char_p]
    lib.axon_stop_nrt_profile.restype = ctypes.c_int64

    @contextlib.contextmanager
    def _hook(output_dir, device_ids):
        import jax

        jax.devices()
        if device_ids:
            ids = (ctypes.c_int64 * len(device_ids))(*device_ids)
            rc = lib.axon_start_nrt_profile(ids, len(device_ids))
        else:
            rc = lib.axon_start_nrt_profile(None, 0)
        if rc != 0:
            raise RuntimeError(f"axon_start_nrt_profile rc={rc}")
        try:
            yield
        finally:
            n = lib.axon_stop_nrt_profile(str(output_dir).encode())
            if n < 0:
                raise RuntimeError(f"axon_stop_nrt_profile rc={n}")

    return _hook


if "antenv.axon_hooks" not in sys.modules:
    _m = types.ModuleType("antenv.axon_hooks")
    _m.get_axon_ntff_profile_hook = lambda: _make_ntff_hook()
    sys.modules["antenv.axon_hooks"] = _m
# --- end shim ---

import numpy as np
import ml_dtypes  # noqa: F401  (bf16 numpy dtype registration)

from concourse import bacc, tile, mybir
from concourse.masks import make_causal_mask, make_identity

B, S, HID = 2, 2048, 2048
HEADS = 16
HD = 128  # head dim
T = B * S  # 4096 tokens
N_CORES = 8
HPC = HEADS // N_CORES  # 2 heads per core
EPS = 1e-6
SCALE = 1.0 / float(np.sqrt(HD))
NEG = -1e9

F32 = mybir.dt.float32
BF16 = mybir.dt.bfloat16

TOKB = 128  # token block (partition dim)
SB = 512  # superblock of tokens for QKV GEMM
N_SB = T // SB  # 8
N_TB = SB // TOKB  # 4
N_CC = HID // 128  # 16 contraction chunks
TOK_SHARD = S // N_CORES  # 256 tokens per (batch, core) after A2A


def _build(apply_mask: bool):
    nc = bacc.Bacc("TRN2", target_bir_lowering=False, debug=False, num_devices=N_CORES)

    inp = nc.dram_tensor("input", [T, HID], F32, kind="ExternalInput").ap()
    wqkv = nc.dram_tensor("qkvw", [HID, 3 * HPC * HD], F32, kind="ExternalInput").ap()
    qkb = nc.dram_tensor("qkb", [128, 2 * HPC], F32, kind="ExternalInput").ap()
    vb = nc.dram_tensor("vb", [1, HPC * HD], F32, kind="ExternalInput").ap()
    ow = nc.dram_tensor("ow", [HID, HID], F32, kind="ExternalInput").ap()
    out = nc.dram_tensor("out", [B * TOK_SHARD, HID], F32, kind="ExternalOutput").ap()
    if apply_mask:
        imask = nc.dram_tensor("imask", [1, B * S], F32, kind="ExternalInput").ap()

    cc_in = [nc.dram_tensor(f"cc_in{b}", [N_CORES, HPC * HD, TOK_SHARD], BF16).ap() for b in range(B)]
    cc_out = [nc.dram_tensor(f"cc_out{b}", [N_CORES, HPC * HD, TOK_SHARD], BF16).ap() for b in range(B)]

    with tile.TileContext(nc) as tc:
        with tc.tile_pool(name="persist", bufs=1) as pers:
            ident = pers.tile([128, 128], BF16)
            make_identity(nc, ident[:])
            causal = pers.tile([128, 128], F32)
            make_causal_mask(nc, causal[:], mask_val=NEG)
            ones1 = pers.tile([1, 128], BF16)
            nc.gpsimd.memset(ones1[:], 1.0)
            eps_t = pers.tile([128, 1], F32)
            nc.gpsimd.memset(eps_t[:], EPS)
            qkb_sb = pers.tile([128, 2 * HPC], F32)
            nc.sync.dma_start(out=qkb_sb[:], in_=qkb[:])
            vbf = pers.tile([1, HPC * HD], F32)
            nc.sync.dma_start(out=vbf[:], in_=vb[:])
            vb_sb = pers.tile([1, HPC * HD], BF16)
            nc.vector.tensor_copy(vb_sb[:], vbf[:])

            qT = pers.tile([128, HPC, T], BF16)  # [d, head, tok]
            kT = pers.tile([128, HPC, T], BF16)
            v_sb = pers.tile([128, T // 128, HPC * HD], BF16)  # [tok128, blk, hcol]

            if apply_mask:
                msk = pers.tile([128, B, S], F32)
                mrow = pers.tile([1, B * S], F32)
                nc.sync.dma_start(out=mrow[:], in_=imask[:])
                for b in range(B):
                    nc.gpsimd.partition_broadcast(msk[:, b, :], mrow[:, b * S : (b + 1) * S])

            # ---------------- Phase A: LN + z^T + QKV GEMM ----------------
            with (
                tc.tile_pool(name="pa_w", bufs=1) as paw,
                tc.tile_pool(name="pa_x", bufs=3) as px,
                tc.tile_pool(name="pa_st", bufs=6) as pst,
                tc.tile_pool(name="pa_z", bufs=3) as pz,
                tc.tile_pool(name="pa_zT", bufs=2) as pzT,
                tc.tile_pool(name="pa_cast", bufs=2) as pcast,
                tc.tile_pool(name="pa_tr", bufs=3, space="PSUM") as ptr,
                tc.tile_pool(name="pa_qk", bufs=2, space="PSUM") as pqk,
                tc.tile_pool(name="pa_v", bufs=2, space="PSUM") as ppv,
            ):
                w_sb = paw.tile([128, N_CC, 3 * HPC * HD], BF16)
                for cc in range(N_CC):
                    wst = pcast.tile([128, 3 * HPC * HD], F32, tag="wst")
                    nc.sync.dma_start(out=wst[:], in_=wqkv[cc * 128 : (cc + 1) * 128, :])
                    nc.vector.tensor_copy(w_sb[:, cc, :], wst[:])

                for sb in range(N_SB):
                    zT = pzT.tile([128, N_CC, SB], BF16)
                    for tb in range(N_TB):
                        r0 = sb * SB + tb * TOKB
                        x_t = px.tile([128, HID], F32)
                        nc.sync.dma_start(out=x_t[:], in_=inp[r0 : r0 + 128, :])
                        bn = pst.tile([128, 4, 6], F32, tag="bn")
                        for c4 in range(4):
                            nc.vector.bn_stats(bn[:, c4, :], x_t[:, c4 * 512 : (c4 + 1) * 512])
                        mv = pst.tile([128, 2], F32, tag="mv")
                        nc.vector.bn_aggr(mv[:], bn[:])
                        sd = pst.tile([128, 1], F32, tag="sd")
                        nc.scalar.activation(sd[:], mv[:, 1:2], mybir.ActivationFunctionType.Sqrt, bias=eps_t[:])
                        istd = pst.tile([128, 1], F32, tag="istd")
                        nc.vector.reciprocal(istd[:], sd[:])
                        z_t = pz.tile([128, HID], BF16)
                        nc.vector.tensor_scalar(
                            out=z_t[:],
                            in0=x_t[:],
                            scalar1=mv[:, 0:1],
                            scalar2=istd[:],
                            op0=mybir.AluOpType.subtract,
                            op1=mybir.AluOpType.mult,
                        )
                        for cc in range(N_CC):
                            ps_t = ptr.tile([128, 128], BF16)
                            nc.tensor.transpose(ps_t[:], z_t[:, cc * 128 : (cc + 1) * 128], ident[:])
                            if cc % 2 == 0:
                                nc.scalar.copy(zT[:, cc, tb * TOKB : tb * TOKB + 128], ps_t[:])
                            else:
                                nc.vector.tensor_copy(zT[:, cc, tb * TOKB : tb * TOKB + 128], ps_t[:])

                    # Q^T, K^T for this superblock (transposed GEMM)
                    for h in range(HPC):
                        for which, base, bias_col, dst in (
                            ("q", 0, h, qT),
                            ("k", HPC * HD, HPC + h, kT),
                        ):
                            psq = pqk.tile([128, SB], F32)
                            for cc in range(N_CC):
                                nc.tensor.matmul(
                                    psq[:],
                                    w_sb[:, cc, base + h * HD : base + (h + 1) * HD],
                                    zT[:, cc, :],
                                    start=(cc == 0),
                                    stop=(cc == N_CC - 1),
                                )
                            nc.vector.tensor_scalar_add(
                                dst[:, h, sb * SB : (sb + 1) * SB], psq[:], qkb_sb[:, bias_col : bias_col + 1]
                            )
                    # V natural
                    for tb in range(N_TB):
                        psv = ppv.tile([128, HPC * HD], F32)
                        for cc in range(N_CC):
                            nc.tensor.matmul(
                                psv[:],
                                zT[:, cc, tb * TOKB : tb * TOKB + 128],
                                w_sb[:, cc, 2 * HPC * HD :],
                                start=(cc == 0),
                                stop=False,
                            )
                        nc.tensor.matmul(psv[:], ones1[:], vb_sb[:], start=False, stop=True)
                        if tb % 2 == 0:
                            nc.scalar.copy(v_sb[:, sb * N_TB + tb, :], psv[:])
                        else:
                            nc.vector.tensor_copy(v_sb[:, sb * N_TB + tb, :], psv[:])

            # ------------- Phase B/C: attention, A2A, output GEMM -------------
            with (
                tc.tile_pool(name="pb_ow", bufs=1) as pow_,
                tc.tile_pool(name="pb_cast", bufs=2) as pcast2,
                tc.tile_pool(name="pb_p", bufs=6) as pp,
                tc.tile_pool(name="pb_pT", bufs=6) as ppT,
                tc.tile_pool(name="pb_st", bufs=8) as pbs,
                tc.tile_pool(name="pb_ctx", bufs=3) as pctx,
                tc.tile_pool(name="pb_ctxT", bufs=2) as pcT,
                tc.tile_pool(name="pb_cf", bufs=2) as pcf,
                tc.tile_pool(name="pb_o", bufs=3) as po,
                tc.tile_pool(name="ps_sc", bufs=2, space="PSUM") as pssc,
                tc.tile_pool(name="ps_tr", bufs=2, space="PSUM") as pstr,
                tc.tile_pool(name="ps_ctx", bufs=1, space="PSUM") as psctx,
                tc.tile_pool(name="ps_o", bufs=2, space="PSUM") as pso,
            ):
                ow_sb = pow_.tile([128, N_CC, HID], BF16)
                for cc in range(N_CC):
                    ost = pcast2.tile([128, HID], F32, tag="ost")
                    nc.sync.dma_start(out=ost[:], in_=ow[cc * 128 : (cc + 1) * 128, :])
                    nc.vector.tensor_copy(ow_sb[:, cc, :], ost[:])

                for b in range(B):
                    ctxT = pcT.tile([128, HPC, S], BF16)
                    for h in range(HPC):
                        for qb in range(S // TOKB):
                            span = (qb + 1) * TOKB
                            nkb = (span + 511) // 512
                            p_chunks = []
                            partials = pbs.tile([128, 4], F32, tag="part")
                            for kb in range(nkb):
                                w = min(512, span - kb * 512)
                                ps = pssc.tile([128, 512], F32)
                                nc.tensor.matmul(
                                    ps[:, :w],
                                    qT[:, h, b * S + qb * TOKB : b * S + qb * TOKB + 128],
                                    kT[:, h, b * S + kb * 512 : b * S + kb * 512 + w],
                                    start=True,
                                    stop=True,
                                )
                                if apply_mask:
                                    nc.vector.tensor_add(
                                        ps[:, :w], ps[:, :w], msk[:, b, kb * 512 : kb * 512 + w]
                                    )
                                if kb == nkb - 1:
                                    nc.vector.tensor_add(ps[:, w - 128 : w], ps[:, w - 128 : w], causal[:])
                                p_c = pp.tile([128, 512], BF16, tag="p")
                                nc.scalar.activation(
                                    p_c[:, :w],
                                    ps[:, :w],
                                    mybir.ActivationFunctionType.Exp,
                                    scale=SCALE,
                                    accum_out=partials[:, kb : kb + 1],
                                )
                                p_chunks.append(p_c)
                            rowsum = pbs.tile([128, 1], F32, tag="rs")
                            nc.vector.tensor_reduce(
                                rowsum[:], partials[:, 0:nkb], axis=mybir.AxisListType.X, op=mybir.AluOpType.add
                            )
                            recip = pbs.tile([128, 1], F32, tag="rc")
                            nc.vector.reciprocal(recip[:], rowsum[:])

                            psc = psctx.tile([128, HD], F32)
                            nkc = qb + 1
                            for kc in range(nkc):
                                pt_ps = pstr.tile([128, 128], BF16, tag="trp")
                                nc.tensor.transpose(
                                    pt_ps[:], p_chunks[kc // 4][:, (kc % 4) * 128 : (kc % 4) * 128 + 128], ident[:]
                                )
                                pT_c = ppT.tile([128, 128], BF16, tag="pT")
                                if kc % 2 == 0:
                                    nc.scalar.copy(pT_c[:], pt_ps[:])
                                else:
                                    nc.vector.tensor_copy(pT_c[:], pt_ps[:])
                                nc.tensor.matmul(
                                    psc[:],
                                    pT_c[:],
                                    v_sb[:, b * (S // 128) + kc, h * HD : (h + 1) * HD],
                                    start=(kc == 0),
                                    stop=(kc == nkc - 1),
                                )
                            ctx_t = pctx.tile([128, HD], BF16)
                            nc.scalar.mul(ctx_t[:], psc[:], recip[:])
                            ct_ps = pstr.tile([128, 128], BF16, tag="trp")
                            nc.tensor.transpose(ct_ps[:], ctx_t[:], ident[:])
                            nc.vector.tensor_copy(ctxT[:, h, qb * TOKB : qb * TOKB + 128], ct_ps[:])
                    for j in range(N_CORES):
                        for h in range(HPC):
                            nc.sync.dma_start(
                                out=cc_in[b][j, h * HD : (h + 1) * HD, :],
                                in_=ctxT[:, h, j * TOK_SHARD : (j + 1) * TOK_SHARD],
                            )
                    nc.gpsimd.collective_compute(
                        "AllToAll",
                        mybir.AluOpType.bypass,
                        replica_groups=[list(range(N_CORES))],
                        ins=[cc_in[b][:]],
                        outs=[cc_out[b][:]],
                    )

                # Output GEMM per batch on this core's token shard
                for b in range(B):
                    cf = pcf.tile([128, N_CC, TOK_SHARD], BF16)
                    for cc in range(N_CC):
                        nc.sync.dma_start(
                            out=cf[:, cc, :],
                            in_=cc_out[b][cc // HPC, (cc % HPC) * 128 : (cc % HPC) * 128 + 128, :],
                        )
                    for tb in range(TOK_SHARD // TOKB):
                        for nb in range(HID // 512):
                            pso_t = pso.tile([128, 512], F32)
                            for cc in range(N_CC):
                                nc.tensor.matmul(
                                    pso_t[:],
                                    cf[:, cc, tb * TOKB : tb * TOKB + 128],
                                    ow_sb[:, cc, nb * 512 : (nb + 1) * 512],
                                    start=(cc == 0),
                                    stop=(cc == N_CC - 1),
                                )
                            o_t = po.tile([128, 512], F32)
                            if nb % 2 == 0:
                                nc.scalar.copy(o_t[:], pso_t[:])
                            else:
                                nc.vector.tensor_copy(o_t[:], pso_t[:])
                            nc.sync.dma_start(
                                out=out[b * TOK_SHARD + tb * TOKB : b * TOK_SHARD + tb * TOKB + 128,
                                        nb * 512 : (nb + 1) * 512],
                                in_=o_t[:],
                            )

    nc.compile()
    return nc


_CACHE = {}


def _get_nc(apply_mask: bool):
    if apply_mask not in _CACHE:
        _CACHE[apply_mask] = _build(apply_mask)
    return _CACHE[apply_mask]


def _prep_in_maps(input, input_mask, norm_w, norm_b, attn_qkvw, attn_qkvb, attn_ow):
    x = np.ascontiguousarray(np.asarray(input, dtype=np.float32).reshape(T, HID))
    w = np.asarray(attn_qkvw, dtype=np.float32)
    nw = np.asarray(norm_w, dtype=np.float32)
    nb = np.asarray(norm_b, dtype=np.float32)
    qb_ = np.asarray(attn_qkvb, dtype=np.float32)
    ow = np.ascontiguousarray(np.asarray(attn_ow, dtype=np.float32))
    mask = np.asarray(input_mask, dtype=np.float32).reshape(B, S)

    w_eff = nw[:, None] * w  # fold LN gamma into QKV weight
    b_eff = nb @ w + qb_  # fold LN beta into QKV bias

    apply_mask = bool(np.any(mask != 0.0))
    in_maps = []
    for i in range(N_CORES):
        cols = []
        for part in range(3):  # q, k, v column shards for this core's heads
            c0 = part * HID + i * HPC * HD
            cols.append(w_eff[:, c0 : c0 + HPC * HD])
        wqkv_i = np.ascontiguousarray(np.concatenate(cols, axis=1))

        bq = b_eff[i * HPC * HD : (i + 1) * HPC * HD].reshape(HPC, HD)
        bk = b_eff[HID + i * HPC * HD : HID + (i + 1) * HPC * HD].reshape(HPC, HD)
        qkb_i = np.ascontiguousarray(np.stack([bq[0], bq[1], bk[0], bk[1]], axis=1))  # [128, 4]
        vb_i = np.ascontiguousarray(
            b_eff[2 * HID + i * HPC * HD : 2 * HID + (i + 1) * HPC * HD].reshape(1, HPC * HD)
        )
        m = {"input": x, "qkvw": wqkv_i, "qkb": qkb_i, "vb": vb_i, "ow": ow}
        if apply_mask:
            m["imask"] = np.ascontiguousarray(mask.reshape(1, B * S))
        in_maps.append(m)
    return in_maps, apply_mask


def _run(inputs: dict, trace: bool = False):
    from concourse.bass_utils import run_bass_kernel_spmd

    in_maps, apply_mask = _prep_in_maps(**inputs)
    nc = _get_nc(apply_mask)
    res = run_bass_kernel_spmd(nc, in_maps, list(range(N_CORES)), trace=trace)
    out = np.empty((B, S, HID), dtype=np.float32)
    for j in range(N_CORES):
        o = res.results[j]["out"]
        for b in range(B):
            out[b, j * TOK_SHARD : (j + 1) * TOK_SHARD] = o[b * TOK_SHARD : (b + 1) * TOK_SHARD]
    return out, res


def kernel(**inputs) -> np.ndarray:
    out, _ = _run(inputs, trace=False)
    return out



# revision 3
# speedup vs baseline: 1.1045x; 1.1045x over previous
"""Tensor-parallel DeepSpeed-style self-attention block on 8 TRN2 NeuronCores.

v2 strategy (sharded LN + AllGather z^T, transposed attention, no PE transposes):
  - Host folds LN params into the QKV weight/bias, drops the K bias (it
    cancels in softmax), and pre-casts qkvw / attn_ow to bf16.
  - Each core LayerNorms only ITS 512 tokens, produces z^T via the DMA
    XBAR transpose (no TensorE/DVE involvement), and AllGathers z^T
    (bf16, 2x 1MB) so every core has all 4096 tokens for its head-shard
    QKV GEMM.  The AG is split in two halves to overlap with LN.
  - QKV GEMM computes Q^T,K^T (transposed: [d, tok]) and V (natural).
  - Attention per (batch, head) is computed fully transposed:
      scores^T[k,q] = K @ Q^T (one MM per 128-k block), exp on ScalarE
      straight into p^T (causal diag handled by a 0/1 upper-tri multiply),
      rowsums via ones-matmul into PSUM, ctx^T[d,q] = V^T @ p^T, then a
      fused normalize (reciprocal + partition_broadcast + multiply).
    This eliminates all per-chunk PE transposes and PSUM->SBUF copies.
  - Attention for batch 0 is interleaved into the tail of the QKV GEMM so
    its ScalarE exp time hides under TensorE GEMM work; A2A(b0) hides
    under the last QKV superblocks; attention(b1) interleaves with the
    output GEMM of b0; A2A(b1) hides under output GEMM b0.
  - Output GEMM is token-sharded after an AllToAll of ctx^T per batch.
"""

import sys

if "/opt/trn_rl_repo" not in sys.path:
    sys.path.insert(0, "/opt/trn_rl_repo")

# --- shim antenv.axon_hooks (missing in this image) so trace=True can NTFF-profile ---
import types, ctypes, contextlib


def _make_ntff_hook(so_path="/opt/axon/libaxon_pjrt.so"):
    try:
        lib = ctypes.CDLL(so_path)
    except OSError:
        return None
    if not hasattr(lib, "axon_start_nrt_profile"):
        return None
    lib.axon_start_nrt_profile.argtypes = [ctypes.POINTER(ctypes.c_int64), ctypes.c_size_t]
    lib.axon_start_nrt_profile.restype = ctypes.c_int64
    lib.axon_stop_nrt_profile.argtypes = [ctypes.c_char_p]
    lib.axon_stop_nrt_profile.restype = ctypes.c_int64

    @contextlib.contextmanager
    def _hook(output_dir, device_ids):
        import jax

        jax.devices()
        if device_ids:
            ids = (ctypes.c_int64 * len(device_ids))(*device_ids)
            rc = lib.axon_start_nrt_profile(ids, len(device_ids))
        else:
            rc = lib.axon_start_nrt_profile(None, 0)
        if rc != 0:
            raise RuntimeError(f"axon_start_nrt_profile rc={rc}")
        try:
            yield
        finally:
            n = lib.axon_stop_nrt_profile(str(output_dir).encode())
            if n < 0:
                raise RuntimeError(f"axon_stop_nrt_profile rc={n}")

    return _hook


if "antenv.axon_hooks" not in sys.modules:
    _m = types.ModuleType("antenv.axon_hooks")
    _m.get_axon_ntff_profile_hook = lambda: _make_ntff_hook()
    sys.modules["antenv.axon_hooks"] = _m
# --- end shim ---

import numpy as np
import ml_dtypes  # noqa: F401  (bf16 numpy dtype registration)

from concourse import bacc, tile, mybir
from concourse.masks import make_upper_triangular

B, S, HID = 2, 2048, 2048
HEADS = 16
HD = 128
T = B * S
N_CORES = 8
HPC = HEADS // N_CORES  # 2 heads per core
EPS = 1e-6
SCALE = 1.0 / float(np.sqrt(HD))

F32 = mybir.dt.float32
BF16 = mybir.dt.bfloat16

SHARD = T // N_CORES  # 512 tokens LN'd per core
HALF = SHARD // 2  # 256-token AllGather granule
N_CC = HID // 128  # 16 contraction chunks
TOK_SHARD = S // N_CORES  # 256 tokens per (batch, core) after A2A
QC = 512  # attention q-chunk width
NQC = S // QC  # 4 q-chunks per batch


def _build(apply_mask: bool):
    nc = bacc.Bacc("TRN2", target_bir_lowering=False, debug=False, num_devices=N_CORES)

    xs = nc.dram_tensor("xshard", [SHARD, HID], F32, kind="ExternalInput").ap()
    wq = nc.dram_tensor("qkvw", [HID, 3 * HPC * HD], BF16, kind="ExternalInput").ap()
    qb = nc.dram_tensor("qbias", [128, HPC], F32, kind="ExternalInput").ap()
    vb = nc.dram_tensor("vbias", [1, HPC * HD], F32, kind="ExternalInput").ap()
    owt = nc.dram_tensor("ow", [HID, HID], BF16, kind="ExternalInput").ap()
    out = nc.dram_tensor("out", [B * TOK_SHARD, HID], F32, kind="ExternalOutput").ap()
    if apply_mask:
        imask = nc.dram_tensor("imask", [128, B * (S // 128)], F32, kind="ExternalInput").ap()

    ag_in = [nc.dram_tensor(f"ag_in{i}", [HID, HALF], BF16).ap() for i in range(2)]
    ag_out = [nc.dram_tensor(f"ag_out{i}", [N_CORES * HID, HALF], BF16).ap() for i in range(2)]
    cc_in = [nc.dram_tensor(f"cc_in{b}", [N_CORES, HPC * HD, TOK_SHARD], BF16).ap() for b in range(B)]
    cc_out = [nc.dram_tensor(f"cc_out{b}", [N_CORES, HPC * HD, TOK_SHARD], BF16).ap() for b in range(B)]

    with tile.TileContext(nc) as tc:
        with tc.tile_pool(name="persist", bufs=1) as pers:
            ones1 = pers.tile([128, 1], BF16)
            nc.gpsimd.memset(ones1[:], 1.0)
            eps_t = pers.tile([128, 1], F32)
            nc.gpsimd.memset(eps_t[:], EPS)
            qb_sb = pers.tile([128, HPC], F32)
            nc.sync.dma_start(out=qb_sb[:], in_=qb[:])
            vbf = pers.tile([1, HPC * HD], F32)
            nc.sync.dma_start(out=vbf[:], in_=vb[:])
            vb_bc = pers.tile([128, HPC * HD], F32)
            nc.gpsimd.partition_broadcast(vb_bc[:], vbf[:])
            trif = pers.tile([128, 128], F32)
            make_upper_triangular(nc, trif[:], val=1.0, diag=True)
            tri01 = pers.tile([128, 128], BF16)
            nc.vector.tensor_copy(tri01[:], trif[:])
            if apply_mask:
                msk = pers.tile([128, B * (S // 128)], F32)
                nc.sync.dma_start(out=msk[:], in_=imask[:])

            qT = pers.tile([128, HPC, T], BF16)  # [d, head, tok]
            kT = pers.tile([128, HPC, T], BF16)
            v_sb = pers.tile([128, T // 128, HPC * HD], BF16)  # [tok128, blk, hcol]

            # ---------- attention emitters (transposed formulation) ----------
            def attn_qc(b, qc, ps_sc, ps_ctx, ps_rs, ppT, prb, prs, ctxT):
                nkb = 4 * qc + 4
                ctx_ps = [ps_ctx.tile([128, QC], F32, tag="ctx", name=f"ctx{h}") for h in range(HPC)]
                rs_ps = [ps_rs.tile([1, QC], F32, tag="rs", name=f"rs{h}") for h in range(HPC)]
                for kb in range(nkb - 1, -1, -1):
                    c0 = max(0, (kb - 4 * qc) * 128)
                    w = QC - c0
                    diag = kb >= 4 * qc
                    for h in range(HPC):
                        sc = ps_sc.tile([128, QC], F32, tag="mm")
                        nc.tensor.matmul(
                            sc[:, :w],
                            kT[:, h, b * S + kb * 128 : b * S + kb * 128 + 128],
                            qT[:, h, b * S + qc * QC + c0 : b * S + qc * QC + c0 + w],
                            start=True,
                            stop=True,
                        )
                        pt = ppT.tile([128, QC], BF16, tag="pt")
                        bias = msk[:, b * 16 + kb : b * 16 + kb + 1] if apply_mask else 0.0
                        nc.scalar.activation(
                            pt[:, :w], sc[:, :w], mybir.ActivationFunctionType.Exp,
                            scale=SCALE, bias=bias,
                        )
                        if diag:
                            nc.vector.tensor_mul(pt[:, 0:128], pt[:, 0:128], tri01[:])
                        nc.tensor.matmul(
                            rs_ps[h][0:1, c0:QC], ones1[:], pt[:, :w],
                            start=(kb == nkb - 1), stop=(kb == 0),
                        )
                        nc.tensor.matmul(
                            ctx_ps[h][:, c0:QC],
                            v_sb[:, b * 16 + kb, h * HD : (h + 1) * HD],
                            pt[:, :w],
                            start=(kb == nkb - 1), stop=(kb == 0),
                        )
                for h in range(HPC):
                    rsb = prs.tile([1, QC], F32, tag="rsb")
                    nc.vector.reciprocal(rsb[:], rs_ps[h][:])
                    rbc = prb.tile([128, QC], F32, tag="rbc")
                    nc.gpsimd.partition_broadcast(rbc[:], rsb[:])
                    nc.vector.tensor_mul(
                        ctxT[:, h, qc * QC : (qc + 1) * QC], ctx_ps[h][:], rbc[:]
                    )

            def ship_ctx(b, ctxT):
                for j in range(N_CORES):
                    nc.sync.dma_start(
                        out=cc_in[b][j].rearrange("(h d) w -> d h w", d=128),
                        in_=ctxT[:, :, j * TOK_SHARD : (j + 1) * TOK_SHARD],
                    )
                nc.gpsimd.collective_compute(
                    "AllToAll",
                    mybir.AluOpType.bypass,
                    replica_groups=[list(range(N_CORES))],
                    ins=[cc_in[b][:]],
                    outs=[cc_out[b][:]],
                )

            # ---------------- Phase A + attention(b0) interleaved ----------------
            with (
                tc.tile_pool(name="pb_pT", bufs=6) as ppT,
                tc.tile_pool(name="pb_cT", bufs=2) as pcT,
                tc.tile_pool(name="pb_rb", bufs=3) as prb,
                tc.tile_pool(name="pb_rs_sb", bufs=2) as prs,
                tc.tile_pool(name="ps_mm", bufs=4, space="PSUM") as ps_mm,
                tc.tile_pool(name="ps_ctx", bufs=2, space="PSUM") as ps_ctx,
                tc.tile_pool(name="ps_rs", bufs=2, space="PSUM") as ps_rs,
            ):
                ctxT0 = pcT.tile([128, HPC, S], BF16, tag="ctxT")
                with (
                    tc.tile_pool(name="pa_w", bufs=1) as paw,
                    tc.tile_pool(name="pa_x", bufs=2) as px,
                    tc.tile_pool(name="pa_st", bufs=4) as pst,
                    tc.tile_pool(name="pa_z", bufs=2) as pz,
                    tc.tile_pool(name="pa_zc", bufs=2) as pzc,
                ):
                    w_sb = paw.tile([128, N_CC, 3 * HPC * HD], BF16)
                    nc.sync.dma_start(out=w_sb[:], in_=wq.rearrange("(c p) f -> p c f", p=128))

                    # LN + DMA-XBAR transpose of own 512 tokens; AllGather in halves
                    with tc.tile_pool(name="pa_zT", bufs=1) as pzo:
                        zT_own = pzo.tile([128, N_CC, SHARD], BF16)
                        for tb in range(4):
                            x_t = px.tile([128, HID], F32, tag="x")
                            nc.sync.dma_start(out=x_t[:], in_=xs[tb * 128 : (tb + 1) * 128, :])
                            bn = pst.tile([128, 4, 6], F32, tag="bn")
                            for c4 in range(4):
                                nc.vector.bn_stats(bn[:, c4, :], x_t[:, c4 * 512 : (c4 + 1) * 512])
                            mv = pst.tile([128, 2], F32, tag="mv")
                            nc.vector.bn_aggr(mv[:], bn[:])
                            sd = pst.tile([128, 1], F32, tag="sd")
                            nc.scalar.activation(
                                sd[:], mv[:, 1:2], mybir.ActivationFunctionType.Sqrt, bias=eps_t[:]
                            )
                            istd = pst.tile([128, 1], F32, tag="istd")
                            nc.vector.reciprocal(istd[:], sd[:])
                            z_t = pz.tile([128, HID], BF16, tag="z")
                            nc.vector.tensor_scalar(
                                out=z_t[:],
                                in0=x_t[:],
                                scalar1=mv[:, 0:1],
                                scalar2=istd[:],
                                op0=mybir.AluOpType.subtract,
                                op1=mybir.AluOpType.mult,
                            )
                            nc.sync.dma_start_transpose(
                                out=zT_own[:, :, tb * 128 : (tb + 1) * 128], in_=z_t[:]
                            )
                            if tb % 2 == 1:
                                half = tb // 2
                                nc.sync.dma_start(
                                    out=ag_in[half].rearrange("(c p) w -> p c w", p=128),
                                    in_=zT_own[:, :, half * HALF : (half + 1) * HALF],
                                )
                                nc.gpsimd.collective_compute(
                                    "AllGather",
                                    mybir.AluOpType.bypass,
                                    replica_groups=[list(range(N_CORES))],
                                    ins=[ag_in[half][:]],
                                    outs=[ag_out[half][:]],
                                )

                    # QKV GEMM over 16 (rank, half) 256-token blocks
                    def qkv_hsb(r, half):
                        col0 = r * SHARD + half * HALF  # global token offset
                        zc = pzc.tile([128, N_CC, HALF], BF16, tag="zc")
                        nc.sync.dma_start(
                            out=zc[:],
                            in_=ag_out[half][r * HID : (r + 1) * HID, :].rearrange(
                                "(c p) w -> p c w", p=128
                            ),
                        )
                        for h in range(HPC):
                            psq = ps_mm.tile([128, HALF], F32, tag="mm")
                            for cc in range(N_CC):
                                nc.tensor.matmul(
                                    psq[:],
                                    w_sb[:, cc, h * HD : (h + 1) * HD],
                                    zc[:, cc, :],
                                    start=(cc == 0),
                                    stop=(cc == N_CC - 1),
                                )
                            nc.vector.tensor_scalar_add(
                                qT[:, h, col0 : col0 + HALF], psq[:], qb_sb[:, h : h + 1]
                            )
                            psk = ps_mm.tile([128, HALF], F32, tag="mm")
                            for cc in range(N_CC):
                                nc.tensor.matmul(
                                    psk[:],
                                    w_sb[:, cc, HPC * HD + h * HD : HPC * HD + (h + 1) * HD],
                                    zc[:, cc, :],
                                    start=(cc == 0),
                                    stop=(cc == N_CC - 1),
                                )
                            nc.scalar.copy(kT[:, h, col0 : col0 + HALF], psk[:])
                        for tb2 in range(2):
                            psv = ps_mm.tile([128, HPC * HD], F32, tag="mm")
                            for cc in range(N_CC):
                                nc.tensor.matmul(
                                    psv[:],
                                    zc[:, cc, tb2 * 128 : (tb2 + 1) * 128],
                                    w_sb[:, cc, 2 * HPC * HD :],
                                    start=(cc == 0),
                                    stop=(cc == N_CC - 1),
                                )
                            nc.vector.tensor_tensor(
                                out=v_sb[:, col0 // 128 + tb2, :],
                                in0=psv[:],
                                in1=vb_bc[:],
                                op=mybir.AluOpType.add,
                            )

                    for r in range(4):
                        qkv_hsb(r, 0)
                    for r in range(4):
                        qkv_hsb(r, 1)
                    # batch-0 q/k/v complete; interleave attention(b0) with rest
                    for r in range(4, 8):
                        qkv_hsb(r, 0)
                        attn_qc(0, r - 4, ps_mm, ps_ctx, ps_rs, ppT, prb, prs, ctxT0)
                    qkv_hsb(4, 1)
                    ship_ctx(0, ctxT0)
                    for r in range(5, 8):
                        qkv_hsb(r, 1)

                # ---------- Phase B: attention(b1) + output GEMMs ----------
                with (
                    tc.tile_pool(name="pb_ow", bufs=1) as pow_,
                    tc.tile_pool(name="pb_cf", bufs=2) as pcf,
                    tc.tile_pool(name="pb_o", bufs=2) as po,
                ):
                    ow_sb = pow_.tile([128, N_CC, HID], BF16)
                    for cc in range(N_CC):
                        nc.sync.dma_start(
                            out=ow_sb[:, cc, :], in_=owt[cc * 128 : (cc + 1) * 128, :]
                        )
                    cf0 = pcf.tile([128, N_CC, TOK_SHARD], BF16, tag="cf")
                    nc.sync.dma_start(
                        out=cf0[:], in_=cc_out[0].rearrange("j (h d) w -> d (j h) w", d=128)
                    )

                    def outg_tb(b, cf, tb):
                        o_t = po.tile([128, HID], F32, tag="o")
                        for nb in range(4):
                            pso = ps_mm.tile([128, 512], F32, tag="mm")
                            for cc in range(N_CC):
                                nc.tensor.matmul(
                                    pso[:],
                                    cf[:, cc, tb * 128 : (tb + 1) * 128],
                                    ow_sb[:, cc, nb * 512 : (nb + 1) * 512],
                                    start=(cc == 0),
                                    stop=(cc == N_CC - 1),
                                )
                            if nb % 2 == 0:
                                nc.scalar.copy(o_t[:, nb * 512 : (nb + 1) * 512], pso[:])
                            else:
                                nc.vector.tensor_copy(o_t[:, nb * 512 : (nb + 1) * 512], pso[:])
                        nc.sync.dma_start(
                            out=out[b * TOK_SHARD + tb * 128 : b * TOK_SHARD + (tb + 1) * 128, :],
                            in_=o_t[:],
                        )

                    ctxT1 = pcT.tile([128, HPC, S], BF16, tag="ctxT")
                    attn_qc(1, 0, ps_mm, ps_ctx, ps_rs, ppT, prb, prs, ctxT1)
                    outg_tb(0, cf0, 0)
                    attn_qc(1, 1, ps_mm, ps_ctx, ps_rs, ppT, prb, prs, ctxT1)
                    attn_qc(1, 2, ps_mm, ps_ctx, ps_rs, ppT, prb, prs, ctxT1)
                    attn_qc(1, 3, ps_mm, ps_ctx, ps_rs, ppT, prb, prs, ctxT1)
                    ship_ctx(1, ctxT1)
                    outg_tb(0, cf0, 1)
                    cf1 = pcf.tile([128, N_CC, TOK_SHARD], BF16, tag="cf")
                    nc.sync.dma_start(
                        out=cf1[:], in_=cc_out[1].rearrange("j (h d) w -> d (j h) w", d=128)
                    )
                    outg_tb(1, cf1, 0)
                    outg_tb(1, cf1, 1)

    nc.compile()
    return nc


_CACHE = {}


def _get_nc(apply_mask: bool):
    if apply_mask not in _CACHE:
        _CACHE[apply_mask] = _build(apply_mask)
    return _CACHE[apply_mask]


def _prep_in_maps(input, input_mask, norm_w, norm_b, attn_qkvw, attn_qkvb, attn_ow):
    bf16 = ml_dtypes.bfloat16
    x = np.ascontiguousarray(np.asarray(input, dtype=np.float32).reshape(T, HID))
    w = np.asarray(attn_qkvw, dtype=np.float32)
    nw = np.asarray(norm_w, dtype=np.float32)
    nb = np.asarray(norm_b, dtype=np.float32)
    qb_ = np.asarray(attn_qkvb, dtype=np.float32)
    ow = np.ascontiguousarray(np.asarray(attn_ow, dtype=np.float32).astype(bf16))
    mask = np.asarray(input_mask, dtype=np.float32).reshape(B, S)

    w_eff = nw[:, None] * w  # fold LN gamma into QKV weight
    b_eff = nb @ w + qb_  # fold LN beta into QKV bias

    apply_mask = bool(np.any(mask != 0.0))
    if apply_mask:
        # per-key layout: [128 partitions (k within block), B * 16 key-blocks]
        mprep = np.ascontiguousarray(
            mask.reshape(B, S // 128, 128).transpose(2, 0, 1).reshape(128, B * (S // 128))
        )
    in_maps = []
    for i in range(N_CORES):
        cols = []
        for part in range(3):  # q, k, v column shards for this core's heads
            c0 = part * HID + i * HPC * HD
            cols.append(w_eff[:, c0 : c0 + HPC * HD])
        wqkv_i = np.ascontiguousarray(np.concatenate(cols, axis=1).astype(bf16))

        bq = b_eff[i * HPC * HD : (i + 1) * HPC * HD].reshape(HPC, HD)
        qb_i = np.ascontiguousarray(np.stack([bq[h] for h in range(HPC)], axis=1))  # [128, HPC]
        vb_i = np.ascontiguousarray(
            b_eff[2 * HID + i * HPC * HD : 2 * HID + (i + 1) * HPC * HD].reshape(1, HPC * HD)
        )
        m = {
            "xshard": np.ascontiguousarray(x[i * SHARD : (i + 1) * SHARD]),
            "qkvw": wqkv_i,
            "qbias": qb_i,
            "vbias": vb_i,
            "ow": ow,
        }
        if apply_mask:
            m["imask"] = mprep
        in_maps.append(m)
    return in_maps, apply_mask


def _run(inputs: dict, trace: bool = False):
    from concourse.bass_utils import run_bass_kernel_spmd

    in_maps, apply_mask = _prep_in_maps(**inputs)
    nc = _get_nc(apply_mask)
    res = run_bass_kernel_spmd(nc, in_maps, list(range(N_CORES)), trace=trace)
    out = np.empty((B, S, HID), dtype=np.float32)
    for j in range(N_CORES):
        o = res.results[j]["out"]
        for b in range(B):
            out[b, j * TOK_SHARD : (j + 1) * TOK_SHARD] = o[b * TOK_SHARD : (b + 1) * TOK_SHARD]
    return out, res


def kernel(**inputs) -> np.ndarray:
    out, _ = _run(inputs, trace=False)
    return out
